# revision 6
# baseline (speedup 1.0000x reference)
"""Trainium2 Bass kernel for an attention block (B=4, T=2048, H=512, 8 heads).

Sharding: 8 cores = 4 batches x 2 query-halves. Core c handles batch c//2 and
query rows [1024*(c%2), 1024*(c%2)+1024) with the full 2048-token context.
Host gather is a pure concatenation of per-core [1024, 512] outputs.

Per-core pipeline (all compute on device):
  1. PE-transpose x -> xT (feature-major) for the QKV projection.
  2. QKV projection (fp32r matmuls): kT [d, tk] and q^T [d, tq] feature-major,
     v in natural [tk, d] layout (bf16) with 64 appended ones-columns.
  3. Per head: scores^T [tk, tq] = kT.T @ qT (fp32r), exp on ScalarE (no
     max-subtraction needed: |scores| < ~10 so exp is fp32-safe), then
     P^T.T... i.e. attn^T [d, tq] = v_aug.T @ expP accumulated over tk tiles.
     The ones-columns of v_aug make rows 64..128 of the PSUM result hold the
     softmax denominator Z replicated 64x, so normalization is a plain
     elementwise multiply by reciprocal(Z) with no partition broadcast.
  4. Output projection (fp32r) + bias, DMA out.
"""

import os
import sys

sys.path.insert(0, "/opt/trn_rl_repo")

from contextlib import ExitStack

import numpy as np

import concourse.bass as bass
import concourse.tile as tile
from concourse import bacc, mybir
from concourse.masks import make_identity

F32 = mybir.dt.float32
F32R = mybir.dt.float32r
BF16 = mybir.dt.bfloat16

B, T, H = 4, 2048, 512
HEADS = 8
D = H // HEADS  # 64
TQ = T // 2  # query rows per core
SCALE = float(D) ** -0.5


def r(ap):
    """Reinterpret an fp32 AP as fp32r for full-rate tensor-engine matmuls."""
    return ap.bitcast(F32R)


def build_nc(Tk=T, Tq=TQ):
    """Build the single-core Bass program (same program runs SPMD on 8 cores)."""
    HC = H // 128  # 4 h-chunks of 128
    NT = Tk // 128  # tk tiles
    NKT = Tk // 512  # 512-wide column chunks of the context
    NQT = Tq // 512  # 512-wide column chunks of the query range
    NQP = Tq // 128  # 128-row query tiles (for out-proj)

    nc = bacc.Bacc("TRN2", target_bir_lowering=False, debug=False, num_devices=8)

    x_kv = nc.dram_tensor("x_kv", [Tk, H], F32, kind="ExternalInput").ap()
    x_q = nc.dram_tensor("x_q", [Tq, H], F32, kind="ExternalInput").ap()
    w_qkv = nc.dram_tensor("w_qkv", [H, 3 * H], F32, kind="ExternalInput").ap()
    b_qkv = nc.dram_tensor("b_qkv", [3 * H], F32, kind="ExternalInput").ap()
    w_proj = nc.dram_tensor("w_proj", [H, H], F32, kind="ExternalInput").ap()
    b_proj = nc.dram_tensor("b_proj", [H], F32, kind="ExternalInput").ap()
    out = nc.dram_tensor("out", [Tq, H], F32, kind="ExternalOutput").ap()

    with tile.TileContext(nc) as tc, ExitStack() as ctx:
        # ---- persistent SBUF (lives for the whole kernel) ----
        per = ctx.enter_context(tc.tile_pool(name="persist", bufs=1))
        ident = per.tile([128, 128], F32)
        make_identity(nc, ident)

        w_proj_sb = per.tile([128, HC, H], F32R)
        nc.sync.dma_start(out=w_proj_sb, in_=w_proj.rearrange("(c p) j -> p c j", p=128).bitcast(F32R))
        bq_sb = per.tile([128, HC], F32)
        nc.sync.dma_start(out=bq_sb, in_=b_qkv[0:H].rearrange("(c p) -> p c", p=128))
        bk_sb = per.tile([128, HC], F32)
        nc.sync.dma_start(out=bk_sb, in_=b_qkv[H : 2 * H].rearrange("(c p) -> p c", p=128))
        bv_row = per.tile([1, H], F32)
        nc.sync.dma_start(out=bv_row, in_=b_qkv[2 * H : 3 * H].unsqueeze(0))
        bv_bc = per.tile([128, H], F32)
        nc.gpsimd.partition_broadcast(bv_bc, bv_row)
        bp_row = per.tile([1, H], F32)
        nc.sync.dma_start(out=bp_row, in_=b_proj.unsqueeze(0))
        bp_bc = per.tile([128, H], F32)
        nc.gpsimd.partition_broadcast(bp_bc, bp_row)

        kT_sb = per.tile([128, HC, Tk], F32R)  # kT: partition j (head-major), free tk
        qT_sb = per.tile([128, HC, Tq], F32R)  # qT (pre-scaled): partition j, free tq
        # v natural [tk, d] per head, bf16, with cols D..2D == 1.0 (Z trick)
        v_sb = per.tile([128, NT, HEADS, 2 * D], BF16)
        nc.vector.memset(v_sb[:, :, :, D : 2 * D], 1.0)
        attnT_sb = per.tile([128, HC, Tq], F32R)  # normalized attn^T, partition j

        # ---- phase 1: transposes + QKV projection ----
        with (
            tc.tile_pool(name="qkvw", bufs=1) as qkvw_pool,
            tc.tile_pool(name="xT", bufs=1) as xT_pool,
            tc.tile_pool(name="xstage", bufs=3) as xstage_pool,
            tc.tile_pool(name="tpsum", bufs=2, space="PSUM") as tpsum,
            tc.tile_pool(name="kqps", bufs=4, space="PSUM") as kqps,
            tc.tile_pool(name="vps", bufs=2, space="PSUM") as vps,
        ):
            w_qkv_sb = qkvw_pool.tile([128, HC, 3 * H], F32R)
            nc.sync.dma_start(
                out=w_qkv_sb, in_=w_qkv.rearrange("(c p) j -> p c j", p=128).bitcast(F32R)
            )
            xkvT = xT_pool.tile([128, HC, Tk], F32R)
            xqT = xT_pool.tile([128, HC, Tq], F32R)

            def transpose_in(x_ap, dst, n_tiles, tag):
                for i in range(n_tiles):
                    xs = xstage_pool.tile([128, H], F32, tag=tag)
                    nc.sync.dma_start(out=xs, in_=x_ap[128 * i : 128 * (i + 1), :])
                    for c in range(HC):
                        tp = tpsum.tile([128, 128], F32, tag="tp")
                        nc.tensor.transpose(tp, xs[:, 128 * c : 128 * (c + 1)], ident)
                        nc.vector.tensor_copy(
                            out=dst[:, c, 128 * i : 128 * (i + 1)], in_=tp
                        )

            transpose_in(x_kv, xkvT, NT, "xs_kv")
            transpose_in(x_q, xqT, Tq // 128, "xs_q")

            # K projection -> kT_sb (feature-major), +bias
            for jt in range(HC):
                ps = [kqps.tile([128, 512], F32, tag="kq", name=f"ps_k{jt}_{t}") for t in range(NKT)]
                for c in range(HC):
                    wcol = H + 128 * jt
                    for tt in range(NKT):
                        nc.tensor.matmul(
                            ps[tt],
                            lhsT=w_qkv_sb[:, c, wcol : wcol + 128],
                            rhs=xkvT[:, c, 512 * tt : 512 * (tt + 1)],
                            start=(c == 0),
                            stop=(c == HC - 1),
                        )
                for tt in range(NKT):
                    nc.vector.tensor_scalar(
                        out=kT_sb[:, jt, 512 * tt : 512 * (tt + 1)],
                        in0=ps[tt],
                        scalar1=bk_sb[:, jt : jt + 1],
                        scalar2=None,
                        op0=mybir.AluOpType.add,
                    )

            # Q projection -> qT_sb, +bias, *scale
            for jt in range(HC):
                ps = [kqps.tile([128, 512], F32, tag="kq", name=f"ps_q{jt}_{t}") for t in range(NQT)]
                for c in range(HC):
                    wcol = 128 * jt
                    for tt in range(NQT):
                        nc.tensor.matmul(
                            ps[tt],
                            lhsT=w_qkv_sb[:, c, wcol : wcol + 128],
                            rhs=xqT[:, c, 512 * tt : 512 * (tt + 1)],
                            start=(c == 0),
                            stop=(c == HC - 1),
                        )
                for tt in range(NQT):
                    nc.vector.tensor_scalar(
                        out=qT_sb[:, jt, 512 * tt : 512 * (tt + 1)],
                        in0=ps[tt],
                        scalar1=bq_sb[:, jt : jt + 1],
                        scalar2=SCALE,
                        op0=mybir.AluOpType.add,
                        op1=mybir.AluOpType.mult,
                    )

            # V projection in natural [tk, j] layout, +bias, -> bf16 v_sb
            for i in range(NT):
                pv = vps.tile([128, 512], F32, tag="v")
                for c in range(HC):
                    nc.tensor.matmul(
                        pv,
                        lhsT=xkvT[:, c, 128 * i : 128 * (i + 1)],
                        rhs=w_qkv_sb[:, c, 2 * H : 3 * H],
                        start=(c == 0),
                        stop=(c == HC - 1),
                    )
                nc.vector.tensor_add(
                    out=v_sb[:, i, :, 0:D],
                    in0=pv.rearrange("p (g d) -> p g d", g=HEADS),
                    in1=bv_bc.rearrange("p (g d) -> p g d", g=HEADS),
                )

        # ---- phase 2: attention per head ----
        with (
            tc.tile_pool(name="expp", bufs=3) as expp_pool,
            tc.tile_pool(name="rz", bufs=2) as rz_pool,
            tc.tile_pool(name="scoreps", bufs=2, space="PSUM") as score_ps,
            tc.tile_pool(name="attnps", bufs=4, space="PSUM") as attn_ps,
        ):
            for g in range(HEADS):
                jt, off = g // 2, D * (g % 2)
                kh = kT_sb[off : off + D, jt, :]
                qh = qT_sb[off : off + D, jt, :]
                acc = [attn_ps.tile([128, 512], F32, tag="acc", name=f"acc_{g}_{t}") for t in range(NQT)]
                for i in range(NT):
                    sp = score_ps.tile([128, Tq], F32, tag="sp")
                    for th in range(NQT):
                        nc.tensor.matmul(
                            sp[:, 512 * th : 512 * (th + 1)],
                            lhsT=kh[:, 128 * i : 128 * (i + 1)],
                            rhs=qh[:, 512 * th : 512 * (th + 1)],
                            start=True,
                            stop=True,
                        )
                    ep = expp_pool.tile([128, Tq], BF16, tag="ep")
                    nc.scalar.activation(ep, sp, mybir.ActivationFunctionType.Exp)
                    for th in range(NQT):
                        nc.tensor.matmul(
                            acc[th],
                            lhsT=v_sb[:, i, g, :],
                            rhs=ep[:, 512 * th : 512 * (th + 1)],
                            start=(i == 0),
                            stop=(i == NT - 1),
                        )
                # rows 0..D hold sum(P*v); rows D..2D hold Z replicated D times
                for th in range(NQT):
                    rz = rz_pool.tile([D, 512], F32, tag="rz")
                    nc.vector.reciprocal(out=rz, in_=acc[th][D : 2 * D, :])
                    nc.vector.tensor_mul(
                        out=attnT_sb[off : off + D, jt, 512 * th : 512 * (th + 1)],
                        in0=acc[th][0:D, :],
                        in1=rz,
                    )

        # ---- phase 3: output projection ----
        with (
            tc.tile_pool(name="ostage", bufs=3) as ostage_pool,
            tc.tile_pool(name="ops", bufs=2, space="PSUM") as ops_pool,
        ):
            for i in range(NQP):
                po = ops_pool.tile([128, H], F32, tag="po")
                for c in range(HC):
                    nc.tensor.matmul(
                        po,
                        lhsT=attnT_sb[:, c, 128 * i : 128 * (i + 1)],
                        rhs=w_proj_sb[:, c, :],
                        start=(c == 0),
                        stop=(c == HC - 1),
                    )
                ot = ostage_pool.tile([128, H], F32, tag="ot")
                nc.vector.tensor_add(out=ot, in0=po, in1=bp_bc)
                nc.sync.dma_start(out=out[128 * i : 128 * (i + 1), :], in_=ot)

    nc.compile()
    return nc


_CACHE = {}


def _get_nc():
    if "nc" not in _CACHE:
        _CACHE["nc"] = build_nc()
    return _CACHE["nc"]


def make_in_maps(x, w_qkv, b_qkv, w_proj, b_proj):
    x = np.ascontiguousarray(np.asarray(x, dtype=np.float32))
    w_qkv = np.ascontiguousarray(np.asarray(w_qkv, dtype=np.float32))
    b_qkv = np.ascontiguousarray(np.asarray(b_qkv, dtype=np.float32))
    w_proj = np.ascontiguousarray(np.asarray(w_proj, dtype=np.float32))
    b_proj = np.ascontiguousarray(np.asarray(b_proj, dtype=np.float32))
    in_maps = []
    for c in range(8):
        b, half = c // 2, c % 2
        in_maps.append(
            {
                "x_kv": x[b],
                "x_q": np.ascontiguousarray(x[b, TQ * half : TQ * (half + 1)]),
                "w_qkv": w_qkv,
                "b_qkv": b_qkv,
                "w_proj": w_proj,
                "b_proj": b_proj,
            }
        )
    return in_maps


def assemble(results):
    full = np.empty((B, T, H), dtype=np.float32)
    for c in range(8):
        b, half = c // 2, c % 2
        full[b, TQ * half : TQ * (half + 1)] = results[c]["out"]
    return full


def kernel(x, w_qkv, b_qkv, w_proj, b_proj):
    from concourse.bass_utils import run_bass_kernel_spmd

    nc = _get_nc()
    in_maps = make_in_maps(x, w_qkv, b_qkv, w_proj, b_proj)
    res = run_bass_kernel_spmd(nc, in_maps, core_ids=list(range(8)))
    return assemble(res.results)


# revision 9
# speedup vs baseline: 1.0175x; 1.0175x over previous
"""Trainium2 Bass kernel for an attention block (B=4, T=2048, H=512, 8 heads).

Sharding: 8 cores = 4 batches x 2 query-halves. Core c handles batch c//2 and
query rows [1024*(c%2), 1024*(c%2)+1024) with the full 2048-token context.
Host gather is a pure concatenation of per-core [1024, 512] outputs.

Per-core pipeline (all compute on device):
  1. PE-transpose x -> xT (feature-major) for the QKV projection.
  2. QKV projection (fp32r matmuls): kT [d, tk] and q^T [d, tq] feature-major,
     v in natural [tk, d] layout (bf16) with 64 appended ones-columns.
  3. Per head: scores^T [tk, tq] = kT.T @ qT (fp32r), exp on ScalarE (no
     max-subtraction needed: |scores| < ~10 so exp is fp32-safe), then
     P^T.T... i.e. attn^T [d, tq] = v_aug.T @ expP accumulated over tk tiles.
     The ones-columns of v_aug make rows 64..128 of the PSUM result hold the
     softmax denominator Z replicated 64x, so normalization is a plain
     elementwise multiply by reciprocal(Z) with no partition broadcast.
  4. Output projection (fp32r) + bias, DMA out.
"""

import os
import sys

sys.path.insert(0, "/opt/trn_rl_repo")

from contextlib import ExitStack

import numpy as np

import concourse.bass as bass
import concourse.tile as tile
from concourse import bacc, mybir
from concourse.masks import make_identity

F32 = mybir.dt.float32
F32R = mybir.dt.float32r
BF16 = mybir.dt.bfloat16

B, T, H = 4, 2048, 512
HEADS = 8
D = H // HEADS  # 64
TQ = T // 2  # query rows per core
SCALE = float(D) ** -0.5


def r(ap):
    """Reinterpret an fp32 AP as fp32r for full-rate tensor-engine matmuls."""
    return ap.bitcast(F32R)


def build_nc(Tk=T, Tq=TQ):
    """Build the single-core Bass program (same program runs SPMD on 8 cores)."""
    HC = H // 128  # 4 h-chunks of 128
    NT = Tk // 128  # tk tiles
    NKT = Tk // 512  # 512-wide column chunks of the context
    NQT = Tq // 512  # 512-wide column chunks of the query range
    NQP = Tq // 128  # 128-row query tiles (for out-proj)

    nc = bacc.Bacc("TRN2", target_bir_lowering=False, debug=False, num_devices=8)

    x_kv = nc.dram_tensor("x_kv", [Tk, H], F32, kind="ExternalInput").ap()
    x_q = nc.dram_tensor("x_q", [Tq, H], F32, kind="ExternalInput").ap()
    w_qkv = nc.dram_tensor("w_qkv", [H, 3 * H], F32, kind="ExternalInput").ap()
    b_qkv = nc.dram_tensor("b_qkv", [3 * H], F32, kind="ExternalInput").ap()
    w_proj = nc.dram_tensor("w_proj", [H, H], F32, kind="ExternalInput").ap()
    b_proj = nc.dram_tensor("b_proj", [H], F32, kind="ExternalInput").ap()
    out = nc.dram_tensor("out", [Tq, H], F32, kind="ExternalOutput").ap()

    with tile.TileContext(nc) as tc, ExitStack() as ctx:
        # ---- persistent SBUF (lives for the whole kernel) ----
        per = ctx.enter_context(tc.tile_pool(name="persist", bufs=1))
        ident = per.tile([128, 128], F32)
        make_identity(nc, ident)

        w_proj_sb = per.tile([128, HC, H], F32R)
        nc.sync.dma_start(out=w_proj_sb, in_=w_proj.rearrange("(c p) j -> p c j", p=128).bitcast(F32R))
        bq_sb = per.tile([128, HC], F32)
        nc.sync.dma_start(out=bq_sb, in_=b_qkv[0:H].rearrange("(c p) -> p c", p=128))
        bk_sb = per.tile([128, HC], F32)
        nc.sync.dma_start(out=bk_sb, in_=b_qkv[H : 2 * H].rearrange("(c p) -> p c", p=128))
        bv_row = per.tile([1, H], F32)
        nc.sync.dma_start(out=bv_row, in_=b_qkv[2 * H : 3 * H].unsqueeze(0))
        bv_bc = per.tile([128, H], F32)
        nc.gpsimd.partition_broadcast(bv_bc, bv_row)
        bp_row = per.tile([1, H], F32)
        nc.sync.dma_start(out=bp_row, in_=b_proj.unsqueeze(0))
        bp_bc = per.tile([128, H], F32)
        nc.gpsimd.partition_broadcast(bp_bc, bp_row)

        kT_sb = per.tile([128, HC, Tk], F32R)  # kT: partition j (head-major), free tk
        qT_sb = per.tile([128, HC, Tq], F32R)  # qT (pre-scaled): partition j, free tq
        # v natural [tk, d] per head, bf16, with cols D..2D == 1.0 (Z trick)
        v_sb = per.tile([128, NT, HEADS, 2 * D], BF16)
        nc.vector.memset(v_sb[:, :, :, 0:D], 1.0)
        attnT_sb = per.tile([128, HC, Tq], F32R)  # normalized attn^T, partition j

        # ---- phase 1: transposes + QKV projection ----
        with (
            tc.tile_pool(name="qkvw", bufs=1) as qkvw_pool,
            tc.tile_pool(name="xT", bufs=1) as xT_pool,
            tc.tile_pool(name="xstage", bufs=3) as xstage_pool,
            tc.tile_pool(name="tpsum", bufs=2, space="PSUM") as tpsum,
            tc.tile_pool(name="kqps", bufs=4, space="PSUM") as kqps,
            tc.tile_pool(name="vps", bufs=2, space="PSUM") as vps,
        ):
            w_qkv_sb = qkvw_pool.tile([128, HC, 3 * H], F32R)
            nc.sync.dma_start(
                out=w_qkv_sb, in_=w_qkv.rearrange("(c p) j -> p c j", p=128).bitcast(F32R)
            )
            xkvT = xT_pool.tile([128, HC, Tk], F32R)
            xqT = xT_pool.tile([128, HC, Tq], F32R)

            def transpose_in(x_ap, dst, n_tiles, tag):
                for i in range(n_tiles):
                    xs = xstage_pool.tile([128, H], F32, tag=tag)
                    nc.sync.dma_start(out=xs, in_=x_ap[128 * i : 128 * (i + 1), :])
                    for c in range(HC):
                        tp = tpsum.tile([128, 128], F32, tag="tp")
                        nc.tensor.transpose(tp, xs[:, 128 * c : 128 * (c + 1)], ident)
                        nc.vector.tensor_copy(
                            out=dst[:, c, 128 * i : 128 * (i + 1)], in_=tp
                        )

            transpose_in(x_kv, xkvT, NT, "xs_kv")
            transpose_in(x_q, xqT, Tq // 128, "xs_q")

            # K projection -> kT_sb (feature-major), +bias
            for jt in range(HC):
                ps = [kqps.tile([128, 512], F32, tag="kq", name=f"ps_k{jt}_{t}") for t in range(NKT)]
                for c in range(HC):
                    wcol = H + 128 * jt
                    for tt in range(NKT):
                        nc.tensor.matmul(
                            ps[tt],
                            lhsT=w_qkv_sb[:, c, wcol : wcol + 128],
                            rhs=xkvT[:, c, 512 * tt : 512 * (tt + 1)],
                            start=(c == 0),
                            stop=(c == HC - 1),
                        )
                for tt in range(NKT):
                    nc.vector.tensor_scalar(
                        out=kT_sb[:, jt, 512 * tt : 512 * (tt + 1)],
                        in0=ps[tt],
                        scalar1=bk_sb[:, jt : jt + 1],
                        scalar2=None,
                        op0=mybir.AluOpType.add,
                    )

            # Q projection -> qT_sb, +bias, *scale
            for jt in range(HC):
                ps = [kqps.tile([128, 512], F32, tag="kq", name=f"ps_q{jt}_{t}") for t in range(NQT)]
                for c in range(HC):
                    wcol = 128 * jt
                    for tt in range(NQT):
                        nc.tensor.matmul(
                            ps[tt],
                            lhsT=w_qkv_sb[:, c, wcol : wcol + 128],
                            rhs=xqT[:, c, 512 * tt : 512 * (tt + 1)],
                            start=(c == 0),
                            stop=(c == HC - 1),
                        )
                for tt in range(NQT):
                    nc.vector.tensor_scalar(
                        out=qT_sb[:, jt, 512 * tt : 512 * (tt + 1)],
                        in0=ps[tt],
                        scalar1=bq_sb[:, jt : jt + 1],
                        scalar2=SCALE,
                        op0=mybir.AluOpType.add,
                        op1=mybir.AluOpType.mult,
                    )

            # V projection in natural [tk, j] layout, +bias, -> bf16 v_sb
            for i in range(NT):
                pv = vps.tile([128, 512], F32, tag="v")
                for c in range(HC):
                    nc.tensor.matmul(
                        pv,
                        lhsT=xkvT[:, c, 128 * i : 128 * (i + 1)],
                        rhs=w_qkv_sb[:, c, 2 * H : 3 * H],
                        start=(c == 0),
                        stop=(c == HC - 1),
                    )
                nc.vector.tensor_add(
                    out=v_sb[:, i, :, D : 2 * D],
                    in0=pv.rearrange("p (g d) -> p g d", g=HEADS),
                    in1=bv_bc.rearrange("p (g d) -> p g d", g=HEADS),
                )

        # ---- phase 2: attention per head ----
        with (
            tc.tile_pool(name="expp", bufs=4) as expp_pool,
            tc.tile_pool(name="rz", bufs=2) as rz_pool,
            tc.tile_pool(name="scoreps", bufs=2, space="PSUM") as score_ps,
            tc.tile_pool(name="attnps", bufs=4, space="PSUM") as attn_ps,
        ):
            for g in range(HEADS):
                jt, off = g // 2, D * (g % 2)
                kh = kT_sb[off : off + D, jt, :]
                qh = qT_sb[off : off + D, jt, :]
                acc = [attn_ps.tile([128, 512], F32, tag="acc", name=f"acc_{g}_{t}") for t in range(NQT)]
                for i in range(NT):
                    sp = score_ps.tile([128, Tq], F32, tag="sp")
                    for th in range(NQT):
                        nc.tensor.matmul(
                            sp[:, 512 * th : 512 * (th + 1)],
                            lhsT=kh[:, 128 * i : 128 * (i + 1)],
                            rhs=qh[:, 512 * th : 512 * (th + 1)],
                            start=True,
                            stop=True,
                        )
                    ep = expp_pool.tile([128, Tq], BF16, tag="ep")
                    nc.scalar.activation(ep, sp, mybir.ActivationFunctionType.Exp)
                    for th in range(NQT):
                        nc.tensor.matmul(
                            acc[th],
                            lhsT=v_sb[:, i, g, :],
                            rhs=ep[:, 512 * th : 512 * (th + 1)],
                            start=(i == 0),
                            stop=(i == NT - 1),
                        )
                # rows 0..D hold Z replicated D times; rows D..2D hold sum(P*v)
                for th in range(NQT):
                    rz = rz_pool.tile([D, 512], F32, tag="rz")
                    nc.vector.reciprocal_approx_fast(out=rz, in_=acc[th][0:D, :])
                    nc.vector.tensor_mul(
                        out=attnT_sb[off : off + D, jt, 512 * th : 512 * (th + 1)],
                        in0=acc[th][D : 2 * D, :],
                        in1=rz,
                    )

        # ---- phase 3: output projection ----
        with (
            tc.tile_pool(name="ostage", bufs=3) as ostage_pool,
            tc.tile_pool(name="ops", bufs=2, space="PSUM") as ops_pool,
        ):
            for i in range(NQP):
                po = ops_pool.tile([128, H], F32, tag="po")
                for c in range(HC):
                    nc.tensor.matmul(
                        po,
                        lhsT=attnT_sb[:, c, 128 * i : 128 * (i + 1)],
                        rhs=w_proj_sb[:, c, :],
                        start=(c == 0),
                        stop=(c == HC - 1),
                    )
                ot = ostage_pool.tile([128, H], F32, tag="ot")
                nc.vector.tensor_add(out=ot, in0=po, in1=bp_bc)
                nc.sync.dma_start(out=out[128 * i : 128 * (i + 1), :], in_=ot)

    nc.compile()
    return nc


_CACHE = {}


def _get_nc():
    if "nc" not in _CACHE:
        _CACHE["nc"] = build_nc()
    return _CACHE["nc"]


def make_in_maps(x, w_qkv, b_qkv, w_proj, b_proj):
    x = np.ascontiguousarray(np.asarray(x, dtype=np.float32))
    w_qkv = np.ascontiguousarray(np.asarray(w_qkv, dtype=np.float32))
    b_qkv = np.ascontiguousarray(np.asarray(b_qkv, dtype=np.float32))
    w_proj = np.ascontiguousarray(np.asarray(w_proj, dtype=np.float32))
    b_proj = np.ascontiguousarray(np.asarray(b_proj, dtype=np.float32))
    in_maps = []
    for c in range(8):
        b, half = c // 2, c % 2
        in_maps.append(
            {
                "x_kv": x[b],
                "x_q": np.ascontiguousarray(x[b, TQ * half : TQ * (half + 1)]),
                "w_qkv": w_qkv,
                "b_qkv": b_qkv,
                "w_proj": w_proj,
                "b_proj": b_proj,
            }
        )
    return in_maps


def assemble(results):
    full = np.empty((B, T, H), dtype=np.float32)
    for c in range(8):
        b, half = c // 2, c % 2
        full[b, TQ * half : TQ * (half + 1)] = results[c]["out"]
    return full


def kernel(x, w_qkv, b_qkv, w_proj, b_proj):
    from concourse.bass_utils import run_bass_kernel_spmd

    nc = _get_nc()
    in_maps = make_in_maps(x, w_qkv, b_qkv, w_proj, b_proj)
    res = run_bass_kernel_spmd(nc, in_maps, core_ids=list(range(8)))
    return assemble(res.results)


# revision 11
# speedup vs baseline: 1.0880x; 1.0693x over previous
"""Trainium2 Bass kernel for an attention block (B=4, T=2048, H=512, 8 heads).

Sharding: 8 cores = 4 batches x 2 query-halves. Core c handles batch c//2 and
query rows [1024*(c%2), 1024*(c%2)+1024) with the full 2048-token context.
Host gather is a pure concatenation of per-core [1024, 512] outputs.

All tensor-engine inputs are bf16: on TRN2 the PE streams fp32 operands at
2 cycles/row (fp32_mode=HIGH, which already truncates to ~bf16 precision),
so real bf16 gives identical numerics at twice the throughput. Accumulation
stays fp32 in PSUM.

Per-core pipeline (all compute on device):
  1. Load x, cast to bf16 (DVE), transpose via DMA xbar into xT [h, t].
  2. QKV projection: kT [d, tk] and qT [d, tq] feature-major (bf16), v in
     natural [tk, d] layout with 64 prepended ones-columns (bf16).
  3. Per head: scoresT [tk, tq] = kh.T @ qh, exp on ScalarE (no
     max-subtraction: |scores| < ~10 so fp32 exp is safe), then
     attnT [2d, tq] = v_aug.T @ expP accumulated over tk tiles. The
     ones-columns make PSUM rows 0..64 hold the softmax denominator Z
     replicated 64x, so normalization is reciprocal_approx_fast + an
     elementwise multiply, no partition broadcast.
  4. Output projection + bias, DMA out.
"""

import sys

sys.path.insert(0, "/opt/trn_rl_repo")

from contextlib import ExitStack

import numpy as np

import concourse.bass as bass
import concourse.tile as tile
from concourse import bacc, mybir
from concourse.masks import make_identity

F32 = mybir.dt.float32
BF16 = mybir.dt.bfloat16

B, T, H = 4, 2048, 512
HEADS = 8
D = H // HEADS  # 64
TQ = T // 2  # query rows per core
SCALE = float(D) ** -0.5


def build_nc(Tk=T, Tq=TQ):
    """Build the single-core Bass program (same program runs SPMD on 8 cores)."""
    HC = H // 128  # 4 h-chunks of 128
    NT = Tk // 128  # tk tiles
    NKT = Tk // 512  # 512-wide column chunks of the context
    NQT = Tq // 512  # 512-wide column chunks of the query range
    NQP = Tq // 128  # 128-row query tiles (for out-proj)

    nc = bacc.Bacc("TRN2", target_bir_lowering=False, debug=False, num_devices=8)

    x_kv = nc.dram_tensor("x_kv", [Tk, H], F32, kind="ExternalInput").ap()
    x_q = nc.dram_tensor("x_q", [Tq, H], F32, kind="ExternalInput").ap()
    w_qkv = nc.dram_tensor("w_qkv", [H, 3 * H], F32, kind="ExternalInput").ap()
    b_qkv = nc.dram_tensor("b_qkv", [3 * H], F32, kind="ExternalInput").ap()
    w_proj = nc.dram_tensor("w_proj", [H, H], F32, kind="ExternalInput").ap()
    b_proj = nc.dram_tensor("b_proj", [H], F32, kind="ExternalInput").ap()
    out = nc.dram_tensor("out", [Tq, H], F32, kind="ExternalOutput").ap()

    with tile.TileContext(nc) as tc, ExitStack() as ctx:
        # ---- persistent SBUF (lives for the whole kernel) ----
        per = ctx.enter_context(tc.tile_pool(name="persist", bufs=1))

        w_proj_sb = per.tile([128, HC, H], BF16)
        ident = per.tile([128, 128], BF16)
        make_identity(nc, ident)
        bq_sb = per.tile([128, HC], F32)
        nc.sync.dma_start(out=bq_sb, in_=b_qkv[0:H].rearrange("(c p) -> p c", p=128))
        bk_sb = per.tile([128, HC], F32)
        nc.sync.dma_start(out=bk_sb, in_=b_qkv[H : 2 * H].rearrange("(c p) -> p c", p=128))
        bv_row = per.tile([1, H], F32)
        nc.sync.dma_start(out=bv_row, in_=b_qkv[2 * H : 3 * H].unsqueeze(0))
        bv_bc = per.tile([128, H], F32)
        nc.gpsimd.partition_broadcast(bv_bc, bv_row)
        bp_row = per.tile([1, H], F32)
        nc.sync.dma_start(out=bp_row, in_=b_proj.unsqueeze(0))
        bp_bc = per.tile([128, H], F32)
        nc.gpsimd.partition_broadcast(bp_bc, bp_row)

        kT_sb = per.tile([128, HC, Tk], BF16)  # kT: partition j (head-major), free tk
        qT_sb = per.tile([128, HC, Tq], BF16)  # qT (pre-scaled): partition j, free tq
        # v natural [tk, d] per head; cols 0..D are 1.0 (softmax-Z trick)
        v_sb = per.tile([128, NT, HEADS, 2 * D], BF16)
        nc.vector.memset(v_sb[:, :, :, 0:D], 1.0)
        attnT_sb = per.tile([128, HC, Tq], BF16)  # normalized attn^T, partition j

        # ---- phase 1: load/cast/transpose + QKV projection ----
        with (
            tc.tile_pool(name="qkvw", bufs=1) as qkvw_pool,
            tc.tile_pool(name="xT", bufs=1) as xT_pool,
            tc.tile_pool(name="wstage", bufs=1) as wstage_pool,
            tc.tile_pool(name="xstage", bufs=3) as xstage_pool,
            tc.tile_pool(name="kqps", bufs=4, space="PSUM") as kqps,
            tc.tile_pool(name="vps", bufs=2, space="PSUM") as vps,
            tc.tile_pool(name="tpsum", bufs=2, space="PSUM") as tpsum,
        ):
            wq_st = wstage_pool.tile([128, HC, 3 * H], F32)
            nc.sync.dma_start(out=wq_st, in_=w_qkv.rearrange("(c p) j -> p c j", p=128))
            w_qkv_sb = qkvw_pool.tile([128, HC, 3 * H], BF16)
            nc.vector.tensor_copy(out=w_qkv_sb, in_=wq_st)
            wp_st = wstage_pool.tile([128, HC, H], F32)
            nc.sync.dma_start(out=wp_st, in_=w_proj.rearrange("(c p) j -> p c j", p=128))
            nc.vector.tensor_copy(out=w_proj_sb, in_=wp_st)

            xkvT = xT_pool.tile([128, HC, Tk], BF16)
            xqT = xT_pool.tile([128, HC, Tq], BF16)

            def transpose_in(x_ap, dst, n_tiles, tag):
                for i in range(n_tiles):
                    xs = xstage_pool.tile([128, H], F32, tag=tag, name=f"xs_{tag}_{i}")
                    nc.sync.dma_start(out=xs, in_=x_ap[128 * i : 128 * (i + 1), :])
                    xb = xstage_pool.tile([128, H], BF16, tag=tag + "b", name=f"xb_{tag}_{i}")
                    nc.vector.tensor_copy(out=xb, in_=xs)
                    for c in range(HC):
                        tp = tpsum.tile([128, 128], BF16, tag="tp", name=f"tp_{tag}_{i}_{c}")
                        nc.tensor.transpose(tp, xb[:, 128 * c : 128 * (c + 1)], ident)
                        nc.vector.tensor_copy(
                            out=dst[:, c, 128 * i : 128 * (i + 1)], in_=tp
                        )

            transpose_in(x_kv, xkvT, NT, "xs_kv")
            transpose_in(x_q, xqT, Tq // 128, "xs_q")

            # V projection in natural [tk, j] layout, +bias, -> bf16 v_sb
            for i in range(NT):
                pv = vps.tile([128, 512], F32, tag="v")
                for c in range(HC):
                    nc.tensor.matmul(
                        pv,
                        lhsT=xkvT[:, c, 128 * i : 128 * (i + 1)],
                        rhs=w_qkv_sb[:, c, 2 * H : 3 * H],
                        start=(c == 0),
                        stop=(c == HC - 1),
                    )
                nc.vector.tensor_add(
                    out=v_sb[:, i, :, D : 2 * D],
                    in0=pv.rearrange("p (g d) -> p g d", g=HEADS),
                    in1=bv_bc.rearrange("p (g d) -> p g d", g=HEADS),
                )

            # K projection -> kT_sb (feature-major), +bias
            for jt in range(HC):
                ps = [kqps.tile([128, 512], F32, tag="kq", name=f"ps_k{jt}_{t}") for t in range(NKT)]
                for c in range(HC):
                    wcol = H + 128 * jt
                    for tt in range(NKT):
                        nc.tensor.matmul(
                            ps[tt],
                            lhsT=w_qkv_sb[:, c, wcol : wcol + 128],
                            rhs=xkvT[:, c, 512 * tt : 512 * (tt + 1)],
                            start=(c == 0),
                            stop=(c == HC - 1),
                        )
                for tt in range(NKT):
                    nc.vector.tensor_scalar(
                        out=kT_sb[:, jt, 512 * tt : 512 * (tt + 1)],
                        in0=ps[tt],
                        scalar1=bk_sb[:, jt : jt + 1],
                        scalar2=None,
                        op0=mybir.AluOpType.add,
                    )

            # Q projection -> qT_sb, +bias, *scale
            for jt in range(HC):
                ps = [kqps.tile([128, 512], F32, tag="kq", name=f"ps_q{jt}_{t}") for t in range(NQT)]
                for c in range(HC):
                    wcol = 128 * jt
                    for tt in range(NQT):
                        nc.tensor.matmul(
                            ps[tt],
                            lhsT=w_qkv_sb[:, c, wcol : wcol + 128],
                            rhs=xqT[:, c, 512 * tt : 512 * (tt + 1)],
                            start=(c == 0),
                            stop=(c == HC - 1),
                        )
                for tt in range(NQT):
                    nc.vector.tensor_scalar(
                        out=qT_sb[:, jt, 512 * tt : 512 * (tt + 1)],
                        in0=ps[tt],
                        scalar1=bq_sb[:, jt : jt + 1],
                        scalar2=SCALE,
                        op0=mybir.AluOpType.add,
                        op1=mybir.AluOpType.mult,
                    )

        # ---- phase 2: attention per head ----
        with (
            tc.tile_pool(name="expp", bufs=4) as expp_pool,
            tc.tile_pool(name="rz", bufs=2) as rz_pool,
            tc.tile_pool(name="scoreps", bufs=2, space="PSUM") as score_ps,
            tc.tile_pool(name="attnps", bufs=4, space="PSUM") as attn_ps,
        ):
            for g in range(HEADS):
                jt, off = g // 2, D * (g % 2)
                kh = kT_sb[off : off + D, jt, :]
                qh = qT_sb[off : off + D, jt, :]
                acc = [attn_ps.tile([128, 512], F32, tag="acc", name=f"acc_{g}_{t}") for t in range(NQT)]
                for i in range(NT):
                    sp = score_ps.tile([128, Tq], F32, tag="sp")
                    for th in range(NQT):
                        nc.tensor.matmul(
                            sp[:, 512 * th : 512 * (th + 1)],
                            lhsT=kh[:, 128 * i : 128 * (i + 1)],
                            rhs=qh[:, 512 * th : 512 * (th + 1)],
                            start=True,
                            stop=True,
                        )
                    ep = expp_pool.tile([128, Tq], BF16, tag="ep")
                    nc.scalar.activation(ep, sp, mybir.ActivationFunctionType.Exp)
                    for th in range(NQT):
                        nc.tensor.matmul(
                            acc[th],
                            lhsT=v_sb[:, i, g, :],
                            rhs=ep[:, 512 * th : 512 * (th + 1)],
                            start=(i == 0),
                            stop=(i == NT - 1),
                        )
                # rows 0..D hold Z replicated D times; rows D..2D hold sum(P*v)
                for th in range(NQT):
                    rz = rz_pool.tile([D, 512], F32, tag="rz")
                    nc.vector.reciprocal_approx_fast(out=rz, in_=acc[th][0:D, :])
                    nc.vector.tensor_mul(
                        out=attnT_sb[off : off + D, jt, 512 * th : 512 * (th + 1)],
                        in0=acc[th][D : 2 * D, :],
                        in1=rz,
                    )

        # ---- phase 3: output projection ----
        with (
            tc.tile_pool(name="ostage", bufs=3) as ostage_pool,
            tc.tile_pool(name="ops", bufs=2, space="PSUM") as ops_pool,
        ):
            for i in range(NQP):
                po = ops_pool.tile([128, H], F32, tag="po")
                for c in range(HC):
                    nc.tensor.matmul(
                        po,
                        lhsT=attnT_sb[:, c, 128 * i : 128 * (i + 1)],
                        rhs=w_proj_sb[:, c, :],
                        start=(c == 0),
                        stop=(c == HC - 1),
                    )
                ot = ostage_pool.tile([128, H], F32, tag="ot")
                nc.vector.tensor_add(out=ot, in0=po, in1=bp_bc)
                nc.sync.dma_start(out=out[128 * i : 128 * (i + 1), :], in_=ot)

    nc.compile()
    return nc


_CACHE = {}


def _get_nc():
    if "nc" not in _CACHE:
        _CACHE["nc"] = build_nc()
    return _CACHE["nc"]


def make_in_maps(x, w_qkv, b_qkv, w_proj, b_proj):
    x = np.ascontiguousarray(np.asarray(x, dtype=np.float32))
    w_qkv = np.ascontiguousarray(np.asarray(w_qkv, dtype=np.float32))
    b_qkv = np.ascontiguousarray(np.asarray(b_qkv, dtype=np.float32))
    w_proj = np.ascontiguousarray(np.asarray(w_proj, dtype=np.float32))
    b_proj = np.ascontiguousarray(np.asarray(b_proj, dtype=np.float32))
    in_maps = []
    for c in range(8):
        b, half = c // 2, c % 2
        in_maps.append(
            {
                "x_kv": x[b],
                "x_q": np.ascontiguousarray(x[b, TQ * half : TQ * (half + 1)]),
                "w_qkv": w_qkv,
                "b_qkv": b_qkv,
                "w_proj": w_proj,
                "b_proj": b_proj,
            }
        )
    return in_maps


def assemble(results):
    full = np.empty((B, T, H), dtype=np.float32)
    for c in range(8):
        b, half = c // 2, c % 2
        full[b, TQ * half : TQ * (half + 1)] = results[c]["out"]
    return full


def kernel(x, w_qkv, b_qkv, w_proj, b_proj):
    from concourse.bass_utils import run_bass_kernel_spmd

    nc = _get_nc()
    in_maps = make_in_maps(x, w_qkv, b_qkv, w_proj, b_proj)
    res = run_bass_kernel_spmd(nc, in_maps, core_ids=list(range(8)))
    return assemble(res.results)


# revision 13
# speedup vs baseline: 1.3221x; 1.2152x over previous
"""Trainium2 Bass kernel for an attention block (B=4, T=2048, H=512, 8 heads).

Sharding: 8 cores = 4 batches x 2 query-halves. Core c handles batch c//2 and
query rows [1024*(c%2), 1024*(c%2)+1024) with the full 2048-token context.
Host gather is a pure concatenation of per-core [1024, 512] outputs.

All tensor-engine inputs are bf16: on TRN2 the PE streams fp32 operands at
2 cycles/row (fp32_mode=HIGH, which already truncates to ~bf16 precision),
so real bf16 gives identical numerics at twice the throughput. Accumulation
stays fp32 in PSUM.

The TRN2 PE clock-gate (HAM) only runs the array at 2.4 GHz while the PE
stream is gap-free; any recurring micro-stall re-throttles it to 1.2 GHz.
The attention loop is therefore software-pipelined explicitly (PV runs one
tk-tile behind scores, with the exp in between on ScalarE) and the K/Q
projections for later head-groups are interleaved one matmul at a time into
the attention stream as PE filler, so the PE never idles waiting on exp.

Per-core pipeline (all compute on device):
  1. Load x, cast bf16, PE-transpose into xT [h, t] (bf16, 1 cyc/row).
  2. V projection into natural [tk, d] layout with 64 prepended
     ones-columns; K/Q projection for head-group 0.
  3. Per head: scoresT [tk, tq] = kh.T @ qh, exp on ScalarE (no
     max-subtraction: |scores| < ~10 so fp32 exp is safe), then
     attnT = v_aug.T @ expP accumulated over tk. The ones-columns make
     PSUM rows 0..64 hold the softmax denominator Z replicated 64x, so
     normalization is reciprocal_approx_fast + elementwise multiply.
     K/Q projections for later head-groups run as interleaved filler.
  4. Output projection + bias, DMA out.
"""

import sys

sys.path.insert(0, "/opt/trn_rl_repo")

from contextlib import ExitStack

import numpy as np

import concourse.bass as bass
import concourse.tile as tile
from concourse import bacc, mybir
from concourse.masks import make_identity

F32 = mybir.dt.float32
BF16 = mybir.dt.bfloat16

B, T, H = 4, 2048, 512
HEADS = 8
D = H // HEADS  # 64
TQ = T // 2  # query rows per core
SCALE = float(D) ** -0.5


def build_nc(Tk=T, Tq=TQ):
    """Build the single-core Bass program (same program runs SPMD on 8 cores)."""
    HC = H // 128  # 4 h-chunks of 128
    NT = Tk // 128  # tk tiles
    NKT = Tk // 512  # 512-wide column chunks of the context
    NQT = Tq // 512  # 512-wide column chunks of the query range
    NQP = Tq // 128  # 128-row query tiles (for out-proj)

    nc = bacc.Bacc("TRN2", target_bir_lowering=False, debug=False, num_devices=8)

    x_kv = nc.dram_tensor("x_kv", [Tk, H], F32, kind="ExternalInput").ap()
    x_q = nc.dram_tensor("x_q", [Tq, H], F32, kind="ExternalInput").ap()
    w_qkv = nc.dram_tensor("w_qkv", [H, 3 * H], F32, kind="ExternalInput").ap()
    b_qkv = nc.dram_tensor("b_qkv", [3 * H], F32, kind="ExternalInput").ap()
    w_proj = nc.dram_tensor("w_proj", [H, H], F32, kind="ExternalInput").ap()
    b_proj = nc.dram_tensor("b_proj", [H], F32, kind="ExternalInput").ap()
    out = nc.dram_tensor("out", [Tq, H], F32, kind="ExternalOutput").ap()

    with tile.TileContext(nc) as tc, ExitStack() as ctx:
        # ---- persistent SBUF ----
        per = ctx.enter_context(tc.tile_pool(name="persist", bufs=1))

        w_proj_sb = per.tile([128, HC, H], BF16)
        ident = per.tile([128, 128], BF16)
        make_identity(nc, ident)
        bq_sb = per.tile([128, HC], F32)
        nc.sync.dma_start(out=bq_sb, in_=b_qkv[0:H].rearrange("(c p) -> p c", p=128))
        bk_sb = per.tile([128, HC], F32)
        nc.sync.dma_start(out=bk_sb, in_=b_qkv[H : 2 * H].rearrange("(c p) -> p c", p=128))
        bv_row = per.tile([1, H], F32)
        nc.sync.dma_start(out=bv_row, in_=b_qkv[2 * H : 3 * H].unsqueeze(0))
        bv_bc = per.tile([128, H], F32)
        nc.gpsimd.partition_broadcast(bv_bc, bv_row)
        bp_row = per.tile([1, H], F32)
        nc.sync.dma_start(out=bp_row, in_=b_proj.unsqueeze(0))
        bp_bc = per.tile([128, H], F32)
        nc.gpsimd.partition_broadcast(bp_bc, bp_row)

        # per-head-group (jt) tensors: separate tiles so interleaved writes for
        # a later jt never alias reads of an earlier jt
        kT_sb = [per.tile([128, Tk], BF16, name=f"kT_{j}") for j in range(HC)]
        qT_sb = [per.tile([128, Tq], BF16, name=f"qT_{j}") for j in range(HC)]
        # v natural [tk, d] per head; cols 0..D are 1.0 (softmax-Z trick)
        v_sb = per.tile([128, NT, HEADS, 2 * D], BF16)
        nc.vector.memset(v_sb[:, :, :, 0:D], 1.0)
        attnT_sb = per.tile([128, HC, Tq], BF16)  # normalized attn^T, partition j

        xT_pool = ctx.enter_context(tc.tile_pool(name="xT", bufs=1))
        xkvT = xT_pool.tile([128, HC, Tk], BF16)
        xqT = xT_pool.tile([128, HC, Tq], BF16)
        qkvw_pool = ctx.enter_context(tc.tile_pool(name="qkvw", bufs=1))
        w_qkv_sb = qkvw_pool.tile([128, HC, 3 * H], BF16)

        # ---- phase A: load/cast/transpose + V + K/Q for head-group 0 ----
        with (
            tc.tile_pool(name="wstage", bufs=1) as wstage_pool,
            tc.tile_pool(name="xstage", bufs=3) as xstage_pool,
            tc.tile_pool(name="tpsum", bufs=2, space="PSUM") as tpsum,
            tc.tile_pool(name="qkvps", bufs=4, space="PSUM") as qkvps,
        ):
            wq_st = wstage_pool.tile([128, HC, 3 * H], F32)
            nc.sync.dma_start(out=wq_st, in_=w_qkv.rearrange("(c p) j -> p c j", p=128))
            nc.vector.tensor_copy(out=w_qkv_sb, in_=wq_st)
            wp_st = wstage_pool.tile([128, HC, H], F32)
            nc.sync.dma_start(out=wp_st, in_=w_proj.rearrange("(c p) j -> p c j", p=128))
            nc.vector.tensor_copy(out=w_proj_sb, in_=wp_st)

            def transpose_in(x_ap, dst, n_tiles, tag):
                for i in range(n_tiles):
                    xs = xstage_pool.tile([128, H], F32, tag=tag, name=f"xs_{tag}_{i}")
                    nc.sync.dma_start(out=xs, in_=x_ap[128 * i : 128 * (i + 1), :])
                    xb = xstage_pool.tile([128, H], BF16, tag=tag + "b", name=f"xb_{tag}_{i}")
                    nc.vector.tensor_copy(out=xb, in_=xs)
                    tp = tpsum.tile([128, H], BF16, tag="tp", name=f"tp_{tag}_{i}")
                    for c in range(HC):
                        nc.tensor.transpose(
                            tp[:, 128 * c : 128 * (c + 1)],
                            xb[:, 128 * c : 128 * (c + 1)],
                            ident,
                        )
                    nc.vector.tensor_copy(
                        out=dst[:, :, 128 * i : 128 * (i + 1)],
                        in_=tp.rearrange("p (c t) -> p c t", c=HC),
                    )

            transpose_in(x_kv, xkvT, NT, "xs_kv")
            transpose_in(x_q, xqT, Tq // 128, "xs_q")

            # V projection in natural [tk, j] layout, +bias, -> bf16 v_sb
            for i in range(NT):
                pv = qkvps.tile([128, 512], F32, tag="g", name=f"ps_v{i}")
                for c in range(HC):
                    nc.tensor.matmul(
                        pv,
                        lhsT=xkvT[:, c, 128 * i : 128 * (i + 1)],
                        rhs=w_qkv_sb[:, c, 2 * H : 3 * H],
                        start=(c == 0),
                        stop=(c == HC - 1),
                    )
                nc.vector.tensor_add(
                    out=v_sb[:, i, :, D : 2 * D],
                    in0=pv.rearrange("p (g d) -> p g d", g=HEADS),
                    in1=bv_bc.rearrange("p (g d) -> p g d", g=HEADS),
                )

            def k_group(psum_pool, jt, tt):
                ps = psum_pool.tile([128, 512], F32, tag="g", name=f"ps_k{jt}_{tt}")
                for c in range(HC):
                    yield nc.tensor.matmul(
                        ps,
                        lhsT=w_qkv_sb[:, c, H + 128 * jt : H + 128 * (jt + 1)],
                        rhs=xkvT[:, c, 512 * tt : 512 * (tt + 1)],
                        start=(c == 0),
                        stop=(c == HC - 1),
                    )
                yield nc.vector.tensor_scalar(
                    out=kT_sb[jt][:, 512 * tt : 512 * (tt + 1)],
                    in0=ps,
                    scalar1=bk_sb[:, jt : jt + 1],
                    scalar2=None,
                    op0=mybir.AluOpType.add,
                )

            def q_group(psum_pool, jt, tt):
                ps = psum_pool.tile([128, 512], F32, tag="g", name=f"ps_q{jt}_{tt}")
                for c in range(HC):
                    yield nc.tensor.matmul(
                        ps,
                        lhsT=w_qkv_sb[:, c, 128 * jt : 128 * (jt + 1)],
                        rhs=xqT[:, c, 512 * tt : 512 * (tt + 1)],
                        start=(c == 0),
                        stop=(c == HC - 1),
                    )
                yield nc.vector.tensor_scalar(
                    out=qT_sb[jt][:, 512 * tt : 512 * (tt + 1)],
                    in0=ps,
                    scalar1=bq_sb[:, jt : jt + 1],
                    scalar2=SCALE,
                    op0=mybir.AluOpType.add,
                    op1=mybir.AluOpType.mult,
                )

            # K/Q for head-group 0 up front
            for tt in range(NKT):
                for _ in k_group(qkvps, 0, tt):
                    pass
            for tt in range(NQT):
                for _ in q_group(qkvps, 0, tt):
                    pass

        # ---- phase B: attention with interleaved K/Q filler ----
        with (
            tc.tile_pool(name="expp", bufs=4) as expp_pool,
            tc.tile_pool(name="rz", bufs=2) as rz_pool,
            tc.tile_pool(name="scoreps", bufs=2, space="PSUM") as score_ps,
            tc.tile_pool(name="attnps", bufs=3, space="PSUM") as attn_ps,
            tc.tile_pool(name="kqips", bufs=1, space="PSUM") as kqips,
        ):
            # filler: one step == one matmul (or one DVE drain) of a later
            # K/Q projection group, pumped between attention iterations
            def filler_steps():
                for jt in range(1, HC):
                    for tt in range(NKT):
                        yield from k_group(kqips, jt, tt)
                    for tt in range(NQT):
                        yield from q_group(kqips, jt, tt)

            filler = filler_steps()
            steps_per_jt = (NKT + NQT) * (HC + 1)
            emitted = [0]

            def pump(n):
                for _ in range(n):
                    if next(filler, None) is not None:
                        emitted[0] += 1

            for g in range(HEADS):
                jt, off = g // 2, D * (g % 2)
                # K/Q for this head-group must be fully emitted before any read
                pump(max(0, jt * steps_per_jt - emitted[0]))
                kh = kT_sb[jt][off : off + D, :]
                qh = qT_sb[jt][off : off + D, :]
                acc = [
                    attn_ps.tile([128, 512], F32, tag="acc", name=f"acc_{g}_{t}")
                    for t in range(NQT)
                ]
                sps = []
                eps = []
                for i in range(NT):
                    sp = score_ps.tile([128, Tq], F32, tag="sp", name=f"sp_{g}_{i}")
                    for th in range(NQT):
                        nc.tensor.matmul(
                            sp[:, 512 * th : 512 * (th + 1)],
                            lhsT=kh[:, 128 * i : 128 * (i + 1)],
                            rhs=qh[:, 512 * th : 512 * (th + 1)],
                            start=True,
                            stop=True,
                        )
                    ep = expp_pool.tile([128, Tq], BF16, tag="ep", name=f"ep_{g}_{i}")
                    nc.scalar.activation(ep, sp, mybir.ActivationFunctionType.Exp)
                    eps.append(ep)
                    pump(1)
                    if i > 0:
                        epp = eps[i - 1]
                        for th in range(NQT):
                            nc.tensor.matmul(
                                acc[th],
                                lhsT=v_sb[:, i - 1, g, :],
                                rhs=epp[:, 512 * th : 512 * (th + 1)],
                                start=(i - 1 == 0),
                                stop=False,
                            )
                epp = eps[NT - 1]
                for th in range(NQT):
                    nc.tensor.matmul(
                        acc[th],
                        lhsT=v_sb[:, NT - 1, g, :],
                        rhs=epp[:, 512 * th : 512 * (th + 1)],
                        start=False,
                        stop=True,
                    )
                # rows 0..D hold Z replicated D times; rows D..2D hold sum(P*v)
                for th in range(NQT):
                    rz = rz_pool.tile([D, 512], F32, tag="rz", name=f"rz_{g}_{th}")
                    nc.vector.reciprocal_approx_fast(out=rz, in_=acc[th][0:D, :])
                    nc.vector.tensor_mul(
                        out=attnT_sb[off : off + D, jt, 512 * th : 512 * (th + 1)],
                        in0=acc[th][D : 2 * D, :],
                        in1=rz,
                    )
            pump(1000)  # drain any remaining filler

        # ---- phase C: output projection ----
        with (
            tc.tile_pool(name="ostage", bufs=3) as ostage_pool,
            tc.tile_pool(name="ops", bufs=2, space="PSUM") as ops_pool,
        ):
            for i in range(NQP):
                po = ops_pool.tile([128, H], F32, tag="po", name=f"po_{i}")
                for c in range(HC):
                    nc.tensor.matmul(
                        po,
                        lhsT=attnT_sb[:, c, 128 * i : 128 * (i + 1)],
                        rhs=w_proj_sb[:, c, :],
                        start=(c == 0),
                        stop=(c == HC - 1),
                    )
                ot = ostage_pool.tile([128, H], F32, tag="ot", name=f"ot_{i}")
                nc.vector.tensor_add(out=ot, in0=po, in1=bp_bc)
                nc.sync.dma_start(out=out[128 * i : 128 * (i + 1), :], in_=ot)

    nc.compile()
    return nc


_CACHE = {}


def _get_nc():
    if "nc" not in _CACHE:
        _CACHE["nc"] = build_nc()
    return _CACHE["nc"]


def make_in_maps(x, w_qkv, b_qkv, w_proj, b_proj):
    x = np.ascontiguousarray(np.asarray(x, dtype=np.float32))
    w_qkv = np.ascontiguousarray(np.asarray(w_qkv, dtype=np.float32))
    b_qkv = np.ascontiguousarray(np.asarray(b_qkv, dtype=np.float32))
    w_proj = np.ascontiguousarray(np.asarray(w_proj, dtype=np.float32))
    b_proj = np.ascontiguousarray(np.asarray(b_proj, dtype=np.float32))
    in_maps = []
    for c in range(8):
        b, half = c // 2, c % 2
        in_maps.append(
            {
                "x_kv": x[b],
                "x_q": np.ascontiguousarray(x[b, TQ * half : TQ * (half + 1)]),
                "w_qkv": w_qkv,
                "b_qkv": b_qkv,
                "w_proj": w_proj,
                "b_proj": b_proj,
            }
        )
    return in_maps


def assemble(results):
    full = np.empty((B, T, H), dtype=np.float32)
    for c in range(8):
        b, half = c // 2, c % 2
        full[b, TQ * half : TQ * (half + 1)] = results[c]["out"]
    return full


def kernel(x, w_qkv, b_qkv, w_proj, b_proj):
    from concourse.bass_utils import run_bass_kernel_spmd

    nc = _get_nc()
    in_maps = make_in_maps(x, w_qkv, b_qkv, w_proj, b_proj)
    res = run_bass_kernel_spmd(nc, in_maps, core_ids=list(range(8)))
    return assemble(res.results)


# revision 15
# speedup vs baseline: 1.4907x; 1.1275x over previous
"""Trainium2 Bass kernel for an attention block (B=4, T=2048, H=512, 8 heads).

Sharding: 8 cores = 4 batches x 2 query-halves. Core c handles batch c//2 and
query rows [1024*(c%2), 1024*(c%2)+1024) with the full 2048-token context.
Host gather is a pure concatenation of per-core [1024, 512] outputs.

All tensor-engine inputs are bf16: on TRN2 the PE streams fp32 operands at
2 cycles/row (fp32_mode=HIGH, which already truncates to ~bf16 precision),
so real bf16 gives identical numerics at twice the throughput. Accumulation
stays fp32 in PSUM.

The TRN2 PE clock-gate (HAM) only runs the array at 2.4 GHz while the PE
stream is gap-free; any recurring micro-stall re-throttles it to 1.2 GHz.
The attention loop is therefore software-pipelined explicitly (PV runs one
tk-tile behind scores, with the exp in between on ScalarE) and the K/Q
projections for later head-groups are interleaved one matmul at a time into
the attention stream as PE filler, so the PE never idles waiting on exp.

Per-core pipeline (all compute on device):
  1. Load x, cast bf16, PE-transpose into xT [h, t] (bf16, 1 cyc/row).
  2. V projection into natural [tk, d] layout with 64 prepended
     ones-columns; K/Q projection for head-group 0.
  3. Per head: scoresT [tk, tq] = kh.T @ qh, exp on ScalarE (no
     max-subtraction: |scores| < ~10 so fp32 exp is safe), then
     attnT = v_aug.T @ expP accumulated over tk. The ones-columns make
     PSUM rows 0..64 hold the softmax denominator Z replicated 64x, so
     normalization is reciprocal_approx_fast + elementwise multiply.
     K/Q projections for later head-groups run as interleaved filler.
  4. Output projection + bias, DMA out.
"""

import sys

sys.path.insert(0, "/opt/trn_rl_repo")

from contextlib import ExitStack

import numpy as np

import concourse.bass as bass
import concourse.tile as tile
from concourse import bacc, mybir
from concourse.masks import make_identity

F32 = mybir.dt.float32
BF16 = mybir.dt.bfloat16

B, T, H = 4, 2048, 512
HEADS = 8
D = H // HEADS  # 64
TQ = T // 2  # query rows per core
SCALE = float(D) ** -0.5


def build_nc(Tk=T, Tq=TQ):
    """Build the single-core Bass program (same program runs SPMD on 8 cores)."""
    HC = H // 128  # 4 h-chunks of 128
    NT = Tk // 128  # tk tiles
    NKT = Tk // 512  # 512-wide column chunks of the context
    NQT = Tq // 512  # 512-wide column chunks of the query range
    NQP = Tq // 128  # 128-row query tiles (for out-proj)

    nc = bacc.Bacc("TRN2", target_bir_lowering=False, debug=False, num_devices=8)

    x_kv = nc.dram_tensor("x_kv", [Tk, H], F32, kind="ExternalInput").ap()
    x_q = nc.dram_tensor("x_q", [Tq, H], F32, kind="ExternalInput").ap()
    w_qkv = nc.dram_tensor("w_qkv", [H, 3 * H], F32, kind="ExternalInput").ap()
    b_qkv = nc.dram_tensor("b_qkv", [3 * H], F32, kind="ExternalInput").ap()
    w_proj = nc.dram_tensor("w_proj", [H, H], F32, kind="ExternalInput").ap()
    b_proj = nc.dram_tensor("b_proj", [H], F32, kind="ExternalInput").ap()
    out = nc.dram_tensor("out", [Tq, H], F32, kind="ExternalOutput").ap()

    with tile.TileContext(nc) as tc, ExitStack() as ctx:
        # ---- persistent SBUF ----
        per = ctx.enter_context(tc.tile_pool(name="persist", bufs=1))

        w_proj_sb = per.tile([128, HC, H], BF16)
        ident = per.tile([128, 128], BF16)
        make_identity(nc, ident)
        bq_sb = per.tile([128, HC], F32)
        nc.gpsimd.dma_start(out=bq_sb, in_=b_qkv[0:H].rearrange("(c p) -> p c", p=128))
        bk_sb = per.tile([128, HC], F32)
        nc.gpsimd.dma_start(out=bk_sb, in_=b_qkv[H : 2 * H].rearrange("(c p) -> p c", p=128))
        bv_row = per.tile([1, H], F32)
        nc.gpsimd.dma_start(out=bv_row, in_=b_qkv[2 * H : 3 * H].unsqueeze(0))
        bv_bc = per.tile([128, H], F32)
        nc.gpsimd.partition_broadcast(bv_bc, bv_row)
        bp_row = per.tile([1, H], F32)
        nc.gpsimd.dma_start(out=bp_row, in_=b_proj.unsqueeze(0))
        bp_bc = per.tile([128, H], F32)
        nc.gpsimd.partition_broadcast(bp_bc, bp_row)

        # per-head-group (jt) tensors: separate tiles so interleaved writes for
        # a later jt never alias reads of an earlier jt
        kT_sb = [per.tile([128, Tk], BF16, name=f"kT_{j}") for j in range(HC)]
        qT_sb = [per.tile([128, Tq], BF16, name=f"qT_{j}") for j in range(HC)]
        # v natural [tk, d] per head; cols 0..D are 1.0 (softmax-Z trick)
        v_sb = per.tile([128, NT, HEADS, 2 * D], BF16)
        nc.vector.memset(v_sb[:, :, :, 0:D], 1.0)
        attnT_sb = per.tile([128, HC, Tq], BF16)  # normalized attn^T, partition j

        xT_pool = ctx.enter_context(tc.tile_pool(name="xT", bufs=1))
        xkvT = xT_pool.tile([128, HC, Tk], BF16)
        xqT = xT_pool.tile([128, HC, Tq], BF16)
        qkvw_pool = ctx.enter_context(tc.tile_pool(name="qkvw", bufs=1))
        w_qkv_sb = qkvw_pool.tile([128, HC, 3 * H], BF16)

        # ---- phase A: load/cast/transpose + V + K/Q for head-group 0 ----
        with (
            tc.tile_pool(name="wstage", bufs=1) as wstage_pool,
            tc.tile_pool(name="xstage", bufs=3) as xstage_pool,
            tc.tile_pool(name="tpsum", bufs=2, space="PSUM") as tpsum,
            tc.tile_pool(name="qkvps", bufs=4, space="PSUM") as qkvps,
        ):
            wq_st = wstage_pool.tile([128, HC, 3 * H], F32)
            nc.scalar.dma_start(out=wq_st, in_=w_qkv.rearrange("(c p) j -> p c j", p=128))
            nc.vector.tensor_copy(out=w_qkv_sb, in_=wq_st)
            wp_st = wstage_pool.tile([128, HC, H], F32)
            nc.gpsimd.dma_start(out=wp_st, in_=w_proj.rearrange("(c p) j -> p c j", p=128))
            nc.vector.tensor_copy(out=w_proj_sb, in_=wp_st)

            def transpose_in(x_ap, dst, n_tiles, tag, dma):
                for i in range(n_tiles):
                    xs = xstage_pool.tile([128, H], F32, tag=tag, name=f"xs_{tag}_{i}")
                    dma.dma_start(out=xs, in_=x_ap[128 * i : 128 * (i + 1), :])
                    xb = xstage_pool.tile([128, H], BF16, tag=tag + "b", name=f"xb_{tag}_{i}")
                    nc.vector.tensor_copy(out=xb, in_=xs)
                    tp = tpsum.tile([128, H], BF16, tag="tp", name=f"tp_{tag}_{i}")
                    for c in range(HC):
                        nc.tensor.transpose(
                            tp[:, 128 * c : 128 * (c + 1)],
                            xb[:, 128 * c : 128 * (c + 1)],
                            ident,
                        )
                    nc.vector.tensor_copy(
                        out=dst[:, :, 128 * i : 128 * (i + 1)],
                        in_=tp.rearrange("p (c t) -> p c t", c=HC),
                    )

            transpose_in(x_kv, xkvT, NT, "xs_kv", nc.sync)
            transpose_in(x_q, xqT, Tq // 128, "xs_q", nc.scalar)

            # V projection in natural [tk, j] layout, +bias, -> bf16 v_sb
            for i in range(NT):
                pv = qkvps.tile([128, 512], F32, tag="g", name=f"ps_v{i}")
                for c in range(HC):
                    nc.tensor.matmul(
                        pv,
                        lhsT=xkvT[:, c, 128 * i : 128 * (i + 1)],
                        rhs=w_qkv_sb[:, c, 2 * H : 3 * H],
                        start=(c == 0),
                        stop=(c == HC - 1),
                    )
                nc.vector.tensor_add(
                    out=v_sb[:, i, :, D : 2 * D],
                    in0=pv.rearrange("p (g d) -> p g d", g=HEADS),
                    in1=bv_bc.rearrange("p (g d) -> p g d", g=HEADS),
                )

            def k_group(psum_pool, jt, tt):
                ps = psum_pool.tile([128, 512], F32, tag="g", name=f"ps_k{jt}_{tt}")
                for c in range(HC):
                    yield nc.tensor.matmul(
                        ps,
                        lhsT=w_qkv_sb[:, c, H + 128 * jt : H + 128 * (jt + 1)],
                        rhs=xkvT[:, c, 512 * tt : 512 * (tt + 1)],
                        start=(c == 0),
                        stop=(c == HC - 1),
                    )
                yield nc.vector.tensor_scalar(
                    out=kT_sb[jt][:, 512 * tt : 512 * (tt + 1)],
                    in0=ps,
                    scalar1=bk_sb[:, jt : jt + 1],
                    scalar2=None,
                    op0=mybir.AluOpType.add,
                )

            def q_group(psum_pool, jt, tt):
                ps = psum_pool.tile([128, 512], F32, tag="g", name=f"ps_q{jt}_{tt}")
                for c in range(HC):
                    yield nc.tensor.matmul(
                        ps,
                        lhsT=w_qkv_sb[:, c, 128 * jt : 128 * (jt + 1)],
                        rhs=xqT[:, c, 512 * tt : 512 * (tt + 1)],
                        start=(c == 0),
                        stop=(c == HC - 1),
                    )
                yield nc.vector.tensor_scalar(
                    out=qT_sb[jt][:, 512 * tt : 512 * (tt + 1)],
                    in0=ps,
                    scalar1=bq_sb[:, jt : jt + 1],
                    scalar2=SCALE,
                    op0=mybir.AluOpType.add,
                    op1=mybir.AluOpType.mult,
                )

            # K/Q for head-group 0 up front
            for tt in range(NKT):
                for _ in k_group(qkvps, 0, tt):
                    pass
            for tt in range(NQT):
                for _ in q_group(qkvps, 0, tt):
                    pass

        # ---- phase B: attention with interleaved K/Q filler ----
        with (
            tc.tile_pool(name="expp", bufs=4) as expp_pool,
            tc.tile_pool(name="rz", bufs=2) as rz_pool,
            tc.tile_pool(name="scoreps", bufs=2, space="PSUM") as score_ps,
            tc.tile_pool(name="attnps", bufs=3, space="PSUM") as attn_ps,
            tc.tile_pool(name="kqips", bufs=1, space="PSUM") as kqips,
        ):
            # filler: one step == one matmul (or one DVE drain) of a later
            # K/Q projection group, pumped between attention iterations
            def filler_steps():
                for jt in range(1, HC):
                    for tt in range(NKT):
                        yield from k_group(kqips, jt, tt)
                    for tt in range(NQT):
                        yield from q_group(kqips, jt, tt)

            filler = filler_steps()
            steps_per_jt = (NKT + NQT) * (HC + 1)
            emitted = [0]

            def pump(n):
                for _ in range(n):
                    if next(filler, None) is not None:
                        emitted[0] += 1

            for g in range(HEADS):
                jt, off = g // 2, D * (g % 2)
                # K/Q for this head-group must be fully emitted before any read
                pump(max(0, jt * steps_per_jt - emitted[0]))
                kh = kT_sb[jt][off : off + D, :]
                qh = qT_sb[jt][off : off + D, :]
                acc = [
                    attn_ps.tile([128, 512], F32, tag="acc", name=f"acc_{g}_{t}")
                    for t in range(NQT)
                ]
                sps = []
                eps = []
                for i in range(NT):
                    sp = score_ps.tile([128, Tq], F32, tag="sp", name=f"sp_{g}_{i}")
                    for th in range(NQT):
                        nc.tensor.matmul(
                            sp[:, 512 * th : 512 * (th + 1)],
                            lhsT=kh[:, 128 * i : 128 * (i + 1)],
                            rhs=qh[:, 512 * th : 512 * (th + 1)],
                            start=True,
                            stop=True,
                        )
                    ep = expp_pool.tile([128, Tq], BF16, tag="ep", name=f"ep_{g}_{i}")
                    nc.scalar.activation(ep, sp, mybir.ActivationFunctionType.Exp)
                    eps.append(ep)
                    pump(2 if i < NT // 2 else 1)
                    if i > 0:
                        epp = eps[i - 1]
                        for th in range(NQT):
                            nc.tensor.matmul(
                                acc[th],
                                lhsT=v_sb[:, i - 1, g, :],
                                rhs=epp[:, 512 * th : 512 * (th + 1)],
                                start=(i - 1 == 0),
                                stop=False,
                            )
                epp = eps[NT - 1]
                for th in range(NQT):
                    nc.tensor.matmul(
                        acc[th],
                        lhsT=v_sb[:, NT - 1, g, :],
                        rhs=epp[:, 512 * th : 512 * (th + 1)],
                        start=False,
                        stop=True,
                    )
                # rows 0..D hold Z replicated D times; rows D..2D hold sum(P*v)
                for th in range(NQT):
                    rz = rz_pool.tile([D, 512], F32, tag="rz", name=f"rz_{g}_{th}")
                    nc.vector.reciprocal_approx_fast(out=rz, in_=acc[th][0:D, :])
                    nc.vector.tensor_mul(
                        out=attnT_sb[off : off + D, jt, 512 * th : 512 * (th + 1)],
                        in0=acc[th][D : 2 * D, :],
                        in1=rz,
                    )
            pump(1000)  # drain any remaining filler

        # ---- phase C: output projection ----
        with (
            tc.tile_pool(name="ostage", bufs=3) as ostage_pool,
            tc.tile_pool(name="ops", bufs=2, space="PSUM") as ops_pool,
        ):
            for i in range(NQP):
                po = ops_pool.tile([128, H], F32, tag="po", name=f"po_{i}")
                for c in range(HC):
                    nc.tensor.matmul(
                        po,
                        lhsT=attnT_sb[:, c, 128 * i : 128 * (i + 1)],
                        rhs=w_proj_sb[:, c, :],
                        start=(c == 0),
                        stop=(c == HC - 1),
                    )
                ot = ostage_pool.tile([128, H], F32, tag="ot", name=f"ot_{i}")
                nc.vector.tensor_add(out=ot, in0=po, in1=bp_bc)
                nc.sync.dma_start(out=out[128 * i : 128 * (i + 1), :], in_=ot)

    nc.compile()
    return nc


_CACHE = {}


def _get_nc():
    if "nc" not in _CACHE:
        _CACHE["nc"] = build_nc()
    return _CACHE["nc"]


def make_in_maps(x, w_qkv, b_qkv, w_proj, b_proj):
    x = np.ascontiguousarray(np.asarray(x, dtype=np.float32))
    w_qkv = np.ascontiguousarray(np.asarray(w_qkv, dtype=np.float32))
    b_qkv = np.ascontiguousarray(np.asarray(b_qkv, dtype=np.float32))
    w_proj = np.ascontiguousarray(np.asarray(w_proj, dtype=np.float32))
    b_proj = np.ascontiguousarray(np.asarray(b_proj, dtype=np.float32))
    in_maps = []
    for c in range(8):
        b, half = c // 2, c % 2
        in_maps.append(
            {
                "x_kv": x[b],
                "x_q": np.ascontiguousarray(x[b, TQ * half : TQ * (half + 1)]),
                "w_qkv": w_qkv,
                "b_qkv": b_qkv,
                "w_proj": w_proj,
                "b_proj": b_proj,
            }
        )
    return in_maps


def assemble(results):
    full = np.empty((B, T, H), dtype=np.float32)
    for c in range(8):
        b, half = c // 2, c % 2
        full[b, TQ * half : TQ * (half + 1)] = results[c]["out"]
    return full


def kernel(x, w_qkv, b_qkv, w_proj, b_proj):
    from concourse.bass_utils import run_bass_kernel_spmd

    nc = _get_nc()
    in_maps = make_in_maps(x, w_qkv, b_qkv, w_proj, b_proj)
    res = run_bass_kernel_spmd(nc, in_maps, core_ids=list(range(8)))
    return assemble(res.results)


# revision 17
# speedup vs baseline: 1.4946x; 1.0026x over previous
"""Trainium2 Bass kernel for an attention block (B=4, T=2048, H=512, 8 heads).

Sharding: 8 cores = 4 batches x 2 query-halves. Core c handles batch c//2 and
query rows [1024*(c%2), 1024*(c%2)+1024) with the full 2048-token context.
Host gather is a pure concatenation of per-core [1024, 512] outputs.

All tensor-engine inputs are bf16: on TRN2 the PE streams fp32 operands at
2 cycles/row (fp32_mode=HIGH, which already truncates to ~bf16 precision),
so real bf16 gives identical numerics at twice the throughput. Accumulation
stays fp32 in PSUM.

The TRN2 PE clock-gate (HAM) only runs the array at 2.4 GHz while the PE
stream is gap-free; any recurring micro-stall re-throttles it to 1.2 GHz.
The attention loop is therefore software-pipelined explicitly (PV runs one
tk-tile behind scores, with the exp in between on ScalarE) and the K/Q
projections for later head-groups are interleaved one matmul at a time into
the attention stream as PE filler, so the PE never idles waiting on exp.

Per-core pipeline (all compute on device):
  1. Load x, cast bf16, PE-transpose into xT [h, t] (bf16, 1 cyc/row).
  2. V projection into natural [tk, d] layout with 64 prepended
     ones-columns; K/Q projection for head-group 0.
  3. Per head: scoresT [tk, tq] = kh.T @ qh, exp on ScalarE (no
     max-subtraction: |scores| < ~10 so fp32 exp is safe), then
     attnT = v_aug.T @ expP accumulated over tk. The ones-columns make
     PSUM rows 0..64 hold the softmax denominator Z replicated 64x, so
     normalization is reciprocal_approx_fast + elementwise multiply.
     K/Q projections for later head-groups run as interleaved filler.
  4. Output projection + bias, DMA out.
"""

import sys

sys.path.insert(0, "/opt/trn_rl_repo")

from contextlib import ExitStack

import numpy as np

import concourse.bass as bass
import concourse.tile as tile
from concourse import bacc, mybir
from concourse.masks import make_identity

F32 = mybir.dt.float32
BF16 = mybir.dt.bfloat16

B, T, H = 4, 2048, 512
HEADS = 8
D = H // HEADS  # 64
TQ = T // 2  # query rows per core
SCALE = float(D) ** -0.5


def build_nc(Tk=T, Tq=TQ):
    """Build the single-core Bass program (same program runs SPMD on 8 cores)."""
    HC = H // 128  # 4 h-chunks of 128
    NT = Tk // 128  # tk tiles
    NKT = Tk // 512  # 512-wide column chunks of the context
    NQT = Tq // 512  # 512-wide column chunks of the query range
    NQP = Tq // 128  # 128-row query tiles (for out-proj)

    nc = bacc.Bacc("TRN2", target_bir_lowering=False, debug=False, num_devices=8)

    x_kv = nc.dram_tensor("x_kv", [Tk, H], F32, kind="ExternalInput").ap()
    x_q = nc.dram_tensor("x_q", [Tq, H], F32, kind="ExternalInput").ap()
    w_qkv = nc.dram_tensor("w_qkv", [H, 3 * H], F32, kind="ExternalInput").ap()
    b_qkv = nc.dram_tensor("b_qkv", [3 * H], F32, kind="ExternalInput").ap()
    w_proj = nc.dram_tensor("w_proj", [H, H], F32, kind="ExternalInput").ap()
    b_proj = nc.dram_tensor("b_proj", [H], F32, kind="ExternalInput").ap()
    out = nc.dram_tensor("out", [Tq, H], F32, kind="ExternalOutput").ap()

    with tile.TileContext(nc) as tc, ExitStack() as ctx:
        # ---- persistent SBUF ----
        per = ctx.enter_context(tc.tile_pool(name="persist", bufs=1))

        w_proj_sb = per.tile([128, HC, H], BF16)
        ident = per.tile([128, 128], BF16)
        make_identity(nc, ident)
        bq_sb = per.tile([128, HC], F32)
        nc.sync.dma_start(out=bq_sb, in_=b_qkv[0:H].rearrange("(c p) -> p c", p=128))
        bk_sb = per.tile([128, HC], F32)
        nc.sync.dma_start(out=bk_sb, in_=b_qkv[H : 2 * H].rearrange("(c p) -> p c", p=128))
        bv_row = per.tile([1, H], F32)
        nc.sync.dma_start(out=bv_row, in_=b_qkv[2 * H : 3 * H].unsqueeze(0))
        bv_bc = per.tile([128, H], F32)
        nc.gpsimd.partition_broadcast(bv_bc, bv_row)
        bp_row = per.tile([1, H], F32)
        nc.sync.dma_start(out=bp_row, in_=b_proj.unsqueeze(0))
        bp_bc = per.tile([128, H], F32)
        nc.gpsimd.partition_broadcast(bp_bc, bp_row)

        # per-head-group (jt) tensors: separate tiles so interleaved writes for
        # a later jt never alias reads of an earlier jt
        kT_sb = [per.tile([128, Tk], BF16, name=f"kT_{j}") for j in range(HC)]
        qT_sb = [per.tile([128, Tq], BF16, name=f"qT_{j}") for j in range(HC)]
        # v natural [tk, d] per head; cols 0..D are 1.0 (softmax-Z trick)
        v_sb = per.tile([128, NT, HEADS, 2 * D], BF16)
        nc.gpsimd.memset(v_sb[:, :, :, 0:D], 1.0)
        attnT_sb = per.tile([128, HC, Tq], BF16)  # normalized attn^T, partition j

        xT_pool = ctx.enter_context(tc.tile_pool(name="xT", bufs=1))
        xkvT = xT_pool.tile([128, HC, Tk], BF16)
        xqT = xT_pool.tile([128, HC, Tq], BF16)
        qkvw_pool = ctx.enter_context(tc.tile_pool(name="qkvw", bufs=1))
        w_qkv_sb = qkvw_pool.tile([128, HC, 3 * H], BF16)

        # ---- phase A: load/cast/transpose + V + K/Q for head-group 0 ----
        with (
            tc.tile_pool(name="wstage", bufs=1) as wstage_pool,
            tc.tile_pool(name="xstage", bufs=3) as xstage_pool,
            tc.tile_pool(name="tpsum", bufs=2, space="PSUM") as tpsum,
            tc.tile_pool(name="qkvps", bufs=4, space="PSUM") as qkvps,
        ):
            wq_st = wstage_pool.tile([128, HC, 3 * H], F32)
            nc.scalar.dma_start(out=wq_st, in_=w_qkv.rearrange("(c p) j -> p c j", p=128))
            nc.vector.tensor_copy(out=w_qkv_sb, in_=wq_st)
            wp_st = wstage_pool.tile([128, HC, H], F32)
            nc.scalar.dma_start(out=wp_st, in_=w_proj.rearrange("(c p) j -> p c j", p=128))
            nc.vector.tensor_copy(out=w_proj_sb, in_=wp_st)

            def transpose_in(x_ap, dst, n_tiles, tag, dma):
                for i in range(n_tiles):
                    xs = xstage_pool.tile([128, H], F32, tag=tag, name=f"xs_{tag}_{i}")
                    dma.dma_start(out=xs, in_=x_ap[128 * i : 128 * (i + 1), :])
                    xb = xstage_pool.tile([128, H], BF16, tag=tag + "b", name=f"xb_{tag}_{i}")
                    nc.vector.tensor_copy(out=xb, in_=xs)
                    tp = tpsum.tile([128, H], BF16, tag="tp", name=f"tp_{tag}_{i}")
                    for c in range(HC):
                        nc.tensor.transpose(
                            tp[:, 128 * c : 128 * (c + 1)],
                            xb[:, 128 * c : 128 * (c + 1)],
                            ident,
                        )
                    nc.vector.tensor_copy(
                        out=dst[:, :, 128 * i : 128 * (i + 1)],
                        in_=tp.rearrange("p (c t) -> p c t", c=HC),
                    )

            transpose_in(x_kv, xkvT, NT, "xs_kv", nc.sync)
            transpose_in(x_q, xqT, Tq // 128, "xs_q", nc.scalar)

            # V projection in natural [tk, j] layout, +bias, -> bf16 v_sb
            for i in range(NT):
                pv = qkvps.tile([128, 512], F32, tag="g", name=f"ps_v{i}")
                for c in range(HC):
                    nc.tensor.matmul(
                        pv,
                        lhsT=xkvT[:, c, 128 * i : 128 * (i + 1)],
                        rhs=w_qkv_sb[:, c, 2 * H : 3 * H],
                        start=(c == 0),
                        stop=(c == HC - 1),
                    )
                nc.vector.tensor_add(
                    out=v_sb[:, i, :, D : 2 * D],
                    in0=pv.rearrange("p (g d) -> p g d", g=HEADS),
                    in1=bv_bc.rearrange("p (g d) -> p g d", g=HEADS),
                )

            def k_group(psum_pool, jt, tt):
                ps = psum_pool.tile([128, 512], F32, tag="g", name=f"ps_k{jt}_{tt}")
                for c in range(HC):
                    yield nc.tensor.matmul(
                        ps,
                        lhsT=w_qkv_sb[:, c, H + 128 * jt : H + 128 * (jt + 1)],
                        rhs=xkvT[:, c, 512 * tt : 512 * (tt + 1)],
                        start=(c == 0),
                        stop=(c == HC - 1),
                    )
                yield nc.vector.tensor_scalar(
                    out=kT_sb[jt][:, 512 * tt : 512 * (tt + 1)],
                    in0=ps,
                    scalar1=bk_sb[:, jt : jt + 1],
                    scalar2=None,
                    op0=mybir.AluOpType.add,
                )

            def q_group(psum_pool, jt, tt):
                ps = psum_pool.tile([128, 512], F32, tag="g", name=f"ps_q{jt}_{tt}")
                for c in range(HC):
                    yield nc.tensor.matmul(
                        ps,
                        lhsT=w_qkv_sb[:, c, 128 * jt : 128 * (jt + 1)],
                        rhs=xqT[:, c, 512 * tt : 512 * (tt + 1)],
                        start=(c == 0),
                        stop=(c == HC - 1),
                    )
                yield nc.vector.tensor_scalar(
                    out=qT_sb[jt][:, 512 * tt : 512 * (tt + 1)],
                    in0=ps,
                    scalar1=bq_sb[:, jt : jt + 1],
                    scalar2=SCALE,
                    op0=mybir.AluOpType.add,
                    op1=mybir.AluOpType.mult,
                )

            # K/Q for head-group 0 up front
            for tt in range(NKT):
                for _ in k_group(qkvps, 0, tt):
                    pass
            for tt in range(NQT):
                for _ in q_group(qkvps, 0, tt):
                    pass

        # ---- phase B: attention with interleaved K/Q filler ----
        with (
            tc.tile_pool(name="expp", bufs=4) as expp_pool,
            tc.tile_pool(name="rz", bufs=2) as rz_pool,
            tc.tile_pool(name="scoreps", bufs=2, space="PSUM") as score_ps,
            tc.tile_pool(name="attnps", bufs=3, space="PSUM") as attn_ps,
            tc.tile_pool(name="kqips", bufs=1, space="PSUM") as kqips,
        ):
            # filler: one step == one matmul (or one DVE drain) of a later
            # K/Q projection group, pumped between attention iterations
            def filler_steps():
                for jt in range(1, HC):
                    for tt in range(NKT):
                        yield from k_group(kqips, jt, tt)
                    for tt in range(NQT):
                        yield from q_group(kqips, jt, tt)

            filler = filler_steps()
            steps_per_jt = (NKT + NQT) * (HC + 1)
            emitted = [0]

            def pump(n):
                for _ in range(n):
                    if next(filler, None) is not None:
                        emitted[0] += 1

            # flat (g, i) stream: PV trails scores by one step globally, so the
            # pipeline rolls through head boundaries without a PE/ACT bubble
            accs = {}
            eps = {}

            def normalize(g):
                # rows 0..D hold Z replicated D times; rows D..2D hold sum(P*v)
                jt, off = g // 2, D * (g % 2)
                for th in range(NQT):
                    a = accs[g][th]
                    rz = rz_pool.tile([D, 512], F32, tag="rz", name=f"rz_{g}_{th}")
                    nc.vector.reciprocal_approx_fast(out=rz, in_=a[0:D, :])
                    nc.vector.tensor_mul(
                        out=attnT_sb[off : off + D, jt, 512 * th : 512 * (th + 1)],
                        in0=a[D : 2 * D, :],
                        in1=rz,
                    )

            items = [(g, i) for g in range(HEADS) for i in range(NT)]
            for idx, (g, i) in enumerate(items):
                jt, off = g // 2, D * (g % 2)
                if i == 0:
                    # K/Q for this head-group must be emitted before any read
                    pump(max(0, jt * steps_per_jt - emitted[0]))
                    accs[g] = [
                        attn_ps.tile([128, 512], F32, tag="acc", name=f"acc_{g}_{t}")
                        for t in range(NQT)
                    ]
                kh = kT_sb[jt][off : off + D, :]
                qh = qT_sb[jt][off : off + D, :]
                sp = score_ps.tile([128, Tq], F32, tag="sp", name=f"sp_{g}_{i}")
                for th in range(NQT):
                    nc.tensor.matmul(
                        sp[:, 512 * th : 512 * (th + 1)],
                        lhsT=kh[:, 128 * i : 128 * (i + 1)],
                        rhs=qh[:, 512 * th : 512 * (th + 1)],
                        start=True,
                        stop=True,
                    )
                ep = expp_pool.tile([128, Tq], BF16, tag="ep", name=f"ep_{g}_{i}")
                nc.scalar.activation(ep, sp, mybir.ActivationFunctionType.Exp)
                eps[idx] = ep
                pump(2 if i < NT // 2 else 1)
                if idx > 0:
                    pg, pi = items[idx - 1]
                    epp = eps.pop(idx - 1)
                    for th in range(NQT):
                        nc.tensor.matmul(
                            accs[pg][th],
                            lhsT=v_sb[:, pi, pg, :],
                            rhs=epp[:, 512 * th : 512 * (th + 1)],
                            start=(pi == 0),
                            stop=(pi == NT - 1),
                        )
                    if pi == NT - 1:
                        normalize(pg)
            pg, pi = items[-1]
            epp = eps.pop(len(items) - 1)
            for th in range(NQT):
                nc.tensor.matmul(
                    accs[pg][th],
                    lhsT=v_sb[:, pi, pg, :],
                    rhs=epp[:, 512 * th : 512 * (th + 1)],
                    start=False,
                    stop=True,
                )
            normalize(pg)
            pump(1000)  # drain any remaining filler

        # ---- phase C: output projection ----
        with (
            tc.tile_pool(name="ostage", bufs=3) as ostage_pool,
            tc.tile_pool(name="ops", bufs=2, space="PSUM") as ops_pool,
        ):
            for i in range(NQP):
                po = ops_pool.tile([128, H], F32, tag="po", name=f"po_{i}")
                for c in range(HC):
                    nc.tensor.matmul(
                        po,
                        lhsT=attnT_sb[:, c, 128 * i : 128 * (i + 1)],
                        rhs=w_proj_sb[:, c, :],
                        start=(c == 0),
                        stop=(c == HC - 1),
                    )
                ot = ostage_pool.tile([128, H], F32, tag="ot", name=f"ot_{i}")
                nc.vector.tensor_add(out=ot, in0=po, in1=bp_bc)
                nc.sync.dma_start(out=out[128 * i : 128 * (i + 1), :], in_=ot)

    nc.compile()
    return nc


_CACHE = {}


def _get_nc():
    if "nc" not in _CACHE:
        _CACHE["nc"] = build_nc()
    return _CACHE["nc"]


def make_in_maps(x, w_qkv, b_qkv, w_proj, b_proj):
    x = np.ascontiguousarray(np.asarray(x, dtype=np.float32))
    w_qkv = np.ascontiguousarray(np.asarray(w_qkv, dtype=np.float32))
    b_qkv = np.ascontiguousarray(np.asarray(b_qkv, dtype=np.float32))
    w_proj = np.ascontiguousarray(np.asarray(w_proj, dtype=np.float32))
    b_proj = np.ascontiguousarray(np.asarray(b_proj, dtype=np.float32))
    in_maps = []
    for c in range(8):
        b, half = c // 2, c % 2
        in_maps.append(
            {
                "x_kv": x[b],
                "x_q": np.ascontiguousarray(x[b, TQ * half : TQ * (half + 1)]),
                "w_qkv": w_qkv,
                "b_qkv": b_qkv,
                "w_proj": w_proj,
                "b_proj": b_proj,
            }
        )
    return in_maps


def assemble(results):
    full = np.empty((B, T, H), dtype=np.float32)
    for c in range(8):
        b, half = c // 2, c % 2
        full[b, TQ * half : TQ * (half + 1)] = results[c]["out"]
    return full


def kernel(x, w_qkv, b_qkv, w_proj, b_proj):
    from concourse.bass_utils import run_bass_kernel_spmd

    nc = _get_nc()
    in_maps = make_in_maps(x, w_qkv, b_qkv, w_proj, b_proj)
    res = run_bass_kernel_spmd(nc, in_maps, core_ids=list(range(8)))
    return assemble(res.results)
